# revision 6
# baseline (speedup 1.0000x reference)
"""Trainium2 Bass kernel v2 for nn_CapsuleLayer_4372276707524.

Math per row r=(b,u,n,c), D=16 vector over d (see reference):
  p = a*w;  3 routing iters of c=softmax(l); out=squash(c*p); l += p*out.
Restructured (exact, shift-compensated):
  x2 = beta1*a^2*w^2, y2 = exp(x2-S2SH)
  m = x2*y2;  S2p = sum_d m*y2;  E2 = sum_d y2
  gamma = alpha2/beta1;  x3 = gamma*m;  y3 = exp(x3-S3SH)
  e3 = y2*y3; vbar = w*e3; E3 = sum e3; S3 = a^2*sum vbar^2
  out = (alpha3*a) * vbar
  with alpha = sqrt(S)/(E^2+S) (eps dropped; exact softmax-shift cancel),
  beta1 = sqrt(S1)/(256+S1), S1 = a^2*sum_d w^2.

Layout: big tensors [P=128(u), D=16, K=80(n,c)] d-major bf16; the final out
op writes k-major f32 so the DRAM DMA is linear. Waves of B_CORE=4 batch
units share w[uc] and batch chain/tree ops.

Engines: DVE = TT(2x bf16) products + trees + f32 chains; Act = exps,
squares, sqrts; Pool = m2 product + final scaled transpose; SP = DMAs.
"""

import sys

import numpy as np
import ml_dtypes

if "/opt/trn_rl_repo" not in sys.path:
    sys.path.insert(0, "/opt/trn_rl_repo")

import concourse.bass as bass
import concourse.tile as tile
from concourse import bacc, mybir
from concourse.bass import AP
from concourse.bass_utils import run_bass_kernel_spmd

F32 = mybir.dt.float32
BF16 = mybir.dt.bfloat16
AF = mybir.ActivationFunctionType
OP = mybir.AluOpType

S2SH = 12.0
S3SH = 14.0

B_FULL = 32
N_CORES = 8
B_CORE = B_FULL // N_CORES  # 4
U = 1152
N = 10
C = 8
D = 16
UC = 9
P = 128
K = N * C  # 80
NCD = K * D  # 1280
WB = B_CORE * NCD  # 5120 wave big width
WK = B_CORE * K  # 320 wave chain width

NPBF16 = ml_dtypes.bfloat16


def _bc(ap: AP, axis: int, n: int) -> AP:
    """Insert a broadcast (stride 0) dim at free-axis position `axis`."""
    dims = [list(x) for x in ap.ap]
    dims.insert(axis + 1, [0, n])
    return AP(ap.tensor, ap.offset, dims)


def build_program():
    nc = bacc.Bacc(
        "TRN2", target_bir_lowering=False, debug=False, num_devices=1
    )
    # weights, d-major: [uc, P, D*K]
    w_d = nc.dram_tensor("w", (UC, P, NCD), BF16, kind="ExternalInput").ap()
    w2_d = nc.dram_tensor("w2", (UC, P, NCD), BF16, kind="ExternalInput").ap()
    w2s_d = nc.dram_tensor("w2s", (UC, P, K), F32, kind="ExternalInput").ap()
    # votes a[b,u,c]: [uc, P, B_CORE, C]
    vt_d = nc.dram_tensor("vt", (UC, P, B_CORE * C), BF16,
                          kind="ExternalInput").ap()
    a2_d = nc.dram_tensor("a2", (UC, P, B_CORE * C), F32,
                          kind="ExternalInput").ap()
    # out[uc, p, (b d n c)] bf16 — host permutes to [B,N,U,C,D] f32
    out_d = nc.dram_tensor("out", (UC, P, WB), BF16,
                           kind="ExternalOutput").ap()
    emit(nc, w_d, w2_d, w2s_d, vt_d, a2_d, out_d)
    nc.compile()
    return nc


def emit(nc, w_d, w2_d, w2s_d, vt_d, a2_d, out_d):
    with tile.TileContext(nc) as tc:
        with (
            tc.tile_pool(name="cst", bufs=1) as cpool,
            tc.tile_pool(name="ws", bufs=2) as wspool,     # streamed w/w2
            tc.tile_pool(name="big", bufs=2) as bigp,      # 5 reused big tags
            tc.tile_pool(name="outp", bufs=2) as opool,    # f32 out per-b
            tc.tile_pool(name="sm", bufs=2) as spool,      # chain smalls
            tc.tile_pool(name="tr", bufs=1) as tpool,      # tree temps
        ):
            # ---- constants / per-core resident loads ----
            b2sh = cpool.tile([P, 1], F32, tag="b2sh")
            nc.vector.memset(b2sh[:], -S2SH)
            b3sh = cpool.tile([P, 1], F32, tag="b3sh")
            nc.vector.memset(b3sh[:], -S3SH)

            w2s_sb, vt_sb, a2_sb = [], [], []
            for uc in range(UC):
                w2s_sb.append(cpool.tile([P, K], F32, tag=f"w2s{uc}",
                                         name=f"w2s{uc}"))
                vt_sb.append(cpool.tile([P, B_CORE * C], BF16,
                                        tag=f"vt{uc}", name=f"vt{uc}"))
                a2_sb.append(cpool.tile([P, B_CORE * C], F32,
                                        tag=f"a2_{uc}", name=f"a2_{uc}"))

            def tree2(srcA, srcB, prodA=None):
                """Two d-sums with merged tail stages. Returns (dstA, dstB)
                as [P, WK] f32 APs (slices of one merged tile).
                If prodA=(m4, y4), tree A's stage1 is the pairwise product
                sum of m*y2 halves (S2p) instead of a plain halving."""
                t1 = tpool.tile([P, 2 * B_CORE * 8 * K], BF16, tag="t1AB",
                                name="t1AB")
                t1A = t1[:, 0:B_CORE * 8 * K].rearrange(
                    "p (b d k) -> p b d k", b=B_CORE, d=8)
                t1B = t1[:, B_CORE * 8 * K:].rearrange(
                    "p (b d k) -> p b d k", b=B_CORE, d=8)
                if prodA is None:
                    sA = srcA[:].rearrange(
                        "p (b d k) -> p b d k", b=B_CORE, d=D)
                    nc.vector.tensor_tensor(
                        t1A, sA[:, :, 0:8], sA[:, :, 8:16], OP.add)
                else:
                    m4, y4 = prodA
                    ha = tpool.tile([P, B_CORE * 8 * K], BF16, tag="haT",
                                    name="haT")
                    hav = ha[:].rearrange("p (b d k) -> p b d k",
                                          b=B_CORE, d=8)
                    nc.vector.tensor_tensor(
                        hav, m4[:, :, 0:8], y4[:, :, 0:8], OP.mult)
                    hb = tpool.tile([P, B_CORE * 8 * K], BF16, tag="hbT",
                                    name="hbT")
                    hbv = hb[:].rearrange("p (b d k) -> p b d k",
                                          b=B_CORE, d=8)
                    nc.vector.tensor_tensor(
                        hbv, m4[:, :, 8:16], y4[:, :, 8:16], OP.mult)
                    nc.vector.tensor_tensor(t1A, hav, hbv, OP.add)
                sB = srcB[:].rearrange("p (b d k) -> p b d k", b=B_CORE, d=D)
                nc.vector.tensor_tensor(
                    t1B, sB[:, :, 0:8], sB[:, :, 8:16], OP.add)
                # merged tails over [P, 2, b, d, k]
                t1v = t1[:].rearrange("p (t b d k) -> p t b d k",
                                      t=2, b=B_CORE, d=8)
                t2 = tpool.tile([P, 2 * B_CORE * 4 * K], BF16, tag="t2AB",
                                name="t2AB")
                t2v = t2[:].rearrange("p (t b d k) -> p t b d k",
                                      t=2, b=B_CORE, d=4)
                nc.vector.tensor_tensor(
                    t2v, t1v[:, :, :, 0:4], t1v[:, :, :, 4:8], OP.add)
                t3 = tpool.tile([P, 2 * B_CORE * 2 * K], BF16, tag="t3AB",
                                name="t3AB")
                t3v = t3[:].rearrange("p (t b d k) -> p t b d k",
                                      t=2, b=B_CORE, d=2)
                nc.vector.tensor_tensor(
                    t3v, t2v[:, :, :, 0:2], t2v[:, :, :, 2:4], OP.add)
                dst = spool.tile([P, 2 * WK], F32, tag="dAB", name="dAB")
                dv = dst[:].rearrange("p (t b k) -> p t b k", t=2, b=B_CORE)
                nc.vector.tensor_tensor(
                    dv, t3v[:, :, :, 0], t3v[:, :, :, 1], OP.add)
                return dst[:, 0:WK], dst[:, WK:2 * WK]

            def tree(dst_f32, src, dn=D):
                """Sum over d (outer free dim): src [P, B_CORE*D*K] bf16
                -> dst [P, WK] f32 (shared scratch tags)."""
                s4 = src[:].rearrange("p (b d k) -> p b d k", b=B_CORE, d=dn)
                t1 = tpool.tile([P, B_CORE * 8 * K], BF16, tag="tr8A",
                                name="tr8A")
                t1v = t1[:].rearrange("p (b d k) -> p b d k", b=B_CORE, d=8)
                nc.vector.tensor_tensor(
                    t1v, s4[:, :, 0:8], s4[:, :, 8:16], OP.add)
                t2 = tpool.tile([P, B_CORE * 4 * K], BF16, tag="tr4A",
                                name="tr4A")
                t2v = t2[:].rearrange("p (b d k) -> p b d k", b=B_CORE, d=4)
                nc.vector.tensor_tensor(
                    t2v, t1v[:, :, 0:4], t1v[:, :, 4:8], OP.add)
                t3 = tpool.tile([P, B_CORE * 2 * K], BF16, tag="tr2A",
                                name="tr2A")
                t3v = t3[:].rearrange("p (b d k) -> p b d k", b=B_CORE, d=2)
                nc.vector.tensor_tensor(
                    t3v, t2v[:, :, 0:2], t2v[:, :, 2:4], OP.add)
                dv = dst_f32[:].rearrange("p (b k) -> p b k", b=B_CORE)
                nc.vector.tensor_tensor(
                    dv, t3v[:, :, 0], t3v[:, :, 1], OP.add)

            def wave_stages(uc):
                """Stage closures for one wave (4 b-units of u-chunk uc).
                Big tags reused by liveness:
                  T1: x2(s1-3) x3(s8-9) q3(s13-14)
                  T2: y2(s2-10)
                  T3: m(s3-8) vbar(s11-16)
                  T4: m2(s4-6) y3(s9-10)
                  T5: e3(s10-12)
                """
                st = {}
                a2v = a2_sb[uc][:].rearrange("p (b c) -> p b c", b=B_CORE)
                av = vt_sb[uc][:].rearrange("p (b c) -> p b c", b=B_CORE)

                def big(tag, name):
                    return bigp.tile([P, WB], BF16, tag=tag, name=name)

                def bigv(t):
                    return t[:].rearrange(
                        "p (b d k) -> p b d k", b=B_CORE, d=D)

                def kv(t):
                    return t[:].rearrange("p (b k) -> p b k", b=B_CORE)

                def sm(tag, dt=F32):
                    return spool.tile([P, WK], dt, tag=tag, name=tag)

                def s0():
                    # per-uc constant loads + streamed w2; iter-1 chain
                    nc.sync.dma_start(w2s_sb[uc][:], w2s_d[uc])
                    nc.sync.dma_start(vt_sb[uc][:], vt_d[uc])
                    nc.sync.dma_start(a2_sb[uc][:], a2_d[uc])
                    w2t = wspool.tile([P, NCD], BF16, tag="w2s_t",
                                      name="w2s_t")
                    nc.sync.dma_start(w2t[:], w2_d[uc])
                    st["w2"] = w2t
                    S1 = sm("S1")
                    w2sb_ = _bc(w2s_sb[uc][:].rearrange(
                        "p (n c) -> p n c", n=N), 0, B_CORE)
                    a2b = _bc(a2v, 1, N)
                    S1v = S1[:].rearrange(
                        "p (b n c) -> p b n c", b=B_CORE, n=N)
                    eng0 = nc.vector if uc == 0 else nc.gpsimd
                    eng0.tensor_tensor(S1v, w2sb_, a2b, OP.mult)
                    B1 = sm("scrA")
                    nc.vector.tensor_scalar_add(B1[:], S1[:], 256.0)
                    r1 = sm("scrB")
                    nc.scalar.activation(r1[:], S1[:], AF.Sqrt)
                    ip1 = sm("scrC")
                    nc.vector.reciprocal_approx_fast(ip1[:], B1[:])
                    be1 = sm("scrD")
                    nc.vector.tensor_tensor(be1[:], r1[:], ip1[:], OP.mult)
                    ib1 = sm("ib1")
                    nc.vector.reciprocal_approx_fast(ib1[:], be1[:])
                    bb = sm("bb", BF16)
                    bbv = bb[:].rearrange(
                        "p (b n c) -> p b n c", b=B_CORE, n=N)
                    nc.vector.tensor_tensor(
                        bbv, kv(be1).rearrange("p b (n c) -> p b n c", n=N),
                        _bc(a2v, 1, N), OP.mult)
                    st.update(ib1=ib1, bb=bb)

                def s1():
                    x2 = big("T1", "x2")
                    HB = B_CORE // 2
                    w22 = _bc(st["w2"][:], 0, HB)
                    bbv = st["bb"][:].rearrange("p (b k) -> p b k", b=B_CORE)
                    x2v = bigv(x2)
                    for h in range(2):
                        bs = slice(h * HB, (h + 1) * HB)
                        nc.vector.tensor_tensor(
                            x2v[:, bs],
                            w22.rearrange("p b (d k) -> p b d k", d=D),
                            _bc(bbv[:, bs], 1, D), OP.mult)
                    st["x2"] = x2

                def s2():
                    y2 = big("T2", "y2")
                    H = WB // 2
                    nc.scalar.activation(
                        y2[:, 0:H], st["x2"][:, 0:H], AF.Exp, bias=b2sh[:])
                    nc.scalar.activation(
                        y2[:, H:WB], st["x2"][:, H:WB], AF.Exp, bias=b2sh[:])
                    st["y2"] = y2

                def s3():
                    m = big("T3", "m")
                    H = WB // 2
                    nc.vector.tensor_tensor(
                        m[:, 0:H], st["x2"][:, 0:H], st["y2"][:, 0:H],
                        OP.mult)
                    nc.vector.tensor_tensor(
                        m[:, H:WB], st["x2"][:, H:WB], st["y2"][:, H:WB],
                        OP.mult)
                    st["m"] = m

                def s4():
                    pass

                def s5():
                    m4 = st["m"][:].rearrange(
                        "p (b d k) -> p b d k", b=B_CORE, d=D)
                    y4 = st["y2"][:].rearrange(
                        "p (b d k) -> p b d k", b=B_CORE, d=D)
                    S2p, E2 = tree2(None, st["y2"], prodA=(m4, y4))
                    st["E2"] = E2
                    st["S2p"] = S2p

                def s6():
                    pass

                def s7():
                    S2 = sm("scrA")
                    nc.vector.tensor_tensor(
                        S2[:], st["S2p"], st["ib1"][:], OP.mult)
                    E2q = sm("scrB")
                    nc.scalar.activation(E2q[:], st["E2"], AF.Square)
                    B2 = sm("scrC")
                    nc.vector.tensor_tensor(B2[:], S2[:], E2q[:], OP.add)
                    rS2 = sm("scrD")
                    nc.scalar.activation(rS2[:], S2[:], AF.Sqrt)
                    ip2 = sm("scrE")
                    nc.vector.reciprocal_approx_fast(ip2[:], B2[:])
                    t2 = sm("scrF")
                    nc.vector.tensor_tensor(t2[:], rS2[:], ip2[:], OP.mult)
                    gam = sm("gam", BF16)
                    nc.vector.tensor_tensor(gam[:], t2[:], st["ib1"][:],
                                            OP.mult)
                    st["gam"] = gam

                def s8():
                    # stream w for s11 early
                    wt = wspool.tile([P, NCD], BF16, tag="w_t", name="w_t")
                    nc.sync.dma_start(wt[:], w_d[uc])
                    st["w"] = wt
                    x3 = big("T1", "x3")
                    HB = B_CORE // 2
                    gv = st["gam"][:].rearrange("p (b k) -> p b k", b=B_CORE)
                    x3v = bigv(x3)
                    mv = bigv(st["m"])
                    for h in range(2):
                        bs = slice(h * HB, (h + 1) * HB)
                        nc.vector.tensor_tensor(
                            x3v[:, bs], mv[:, bs], _bc(gv[:, bs], 1, D),
                            OP.mult)
                    st["x3"] = x3

                def s9():
                    y3 = big("T4", "y3")
                    H = WB // 2
                    nc.scalar.activation(
                        y3[:, 0:H], st["x3"][:, 0:H], AF.Exp, bias=b3sh[:])
                    nc.scalar.activation(
                        y3[:, H:WB], st["x3"][:, H:WB], AF.Exp, bias=b3sh[:])
                    st["y3"] = y3

                def s10():
                    e3 = big("T5", "e3")
                    H = WB // 2
                    nc.vector.tensor_tensor(
                        e3[:, 0:H], st["y2"][:, 0:H], st["y3"][:, 0:H],
                        OP.mult)
                    nc.vector.tensor_tensor(
                        e3[:, H:WB], st["y2"][:, H:WB], st["y3"][:, H:WB],
                        OP.mult)
                    st["e3"] = e3

                def s11():
                    vb = big("T3", "vbar")
                    for b in range(B_CORE):
                        sl = slice(b * NCD, (b + 1) * NCD)
                        nc.vector.tensor_tensor(
                            vb[:, sl], st["w"][:], st["e3"][:, sl], OP.mult)
                    st["vb"] = vb

                def s12():
                    q3 = big("T1", "q3")
                    H = WB // 2
                    nc.scalar.activation(q3[:, 0:H], st["vb"][:, 0:H],
                                         AF.Square)
                    nc.scalar.activation(q3[:, H:WB], st["vb"][:, H:WB],
                                         AF.Square)
                    st["q3"] = q3

                def s13():
                    pass

                def s14():
                    S3b, E3 = tree2(st["q3"], st["e3"])
                    st["E3"] = E3
                    st["S3b"] = S3b

                def s15():
                    S3 = sm("scrA")
                    S3v = S3[:].rearrange(
                        "p (b n c) -> p b n c", b=B_CORE, n=N)
                    nc.vector.tensor_tensor(
                        S3v, st["S3b"].rearrange("p (b k) -> p b k", b=B_CORE).rearrange(
                            "p b (n c) -> p b n c", n=N),
                        _bc(a2v, 1, N), OP.mult)
                    E3q = sm("scrB")
                    nc.scalar.activation(E3q[:], st["E3"], AF.Square)
                    B3 = sm("scrC")
                    nc.vector.tensor_tensor(B3[:], S3[:], E3q[:], OP.add)
                    rS3 = sm("scrD")
                    nc.scalar.activation(rS3[:], S3[:], AF.Sqrt)
                    ip3 = sm("scrE")
                    nc.vector.reciprocal_approx_fast(ip3[:], B3[:])
                    t3 = sm("scrF")
                    nc.vector.tensor_tensor(t3[:], rS3[:], ip3[:], OP.mult)
                    a3p = sm("a3p", BF16)
                    a3pv = a3p[:].rearrange(
                        "p (b n c) -> p b n c", b=B_CORE, n=N)
                    nc.vector.tensor_tensor(
                        a3pv, kv(t3).rearrange("p b (n c) -> p b n c", n=N),
                        _bc(av, 1, N), OP.mult)
                    st["a3p"] = a3p

                def s16():
                    ot = opool.tile([P, WB], BF16, tag="out", name="out")
                    HB = B_CORE // 2
                    HW_ = WB // 2
                    a3v = st["a3p"][:].rearrange(
                        "p (b k) -> p b k", b=B_CORE)
                    otv = bigv(ot)
                    vbv = bigv(st["vb"])
                    for h in range(2):
                        bs = slice(h * HB, (h + 1) * HB)
                        nc.vector.tensor_tensor(
                            otv[:, bs], vbv[:, bs], _bc(a3v[:, bs], 1, D),
                            OP.mult)
                        nc.sync.dma_start(
                            out_d[uc][:, h * HW_:(h + 1) * HW_],
                            ot[:, h * HW_:(h + 1) * HW_])

                return [s0, s1, s2, s3, s4, s5, s6, s7, s8, s9, s10, s11,
                        s12, s13, s14, s15, s16]

            # rolling software pipeline: wave i+1 starts OFF stages
            # behind wave i; 2 waves in flight (matches bufs=2 pools)
            OFF = 9
            all_stages = [wave_stages(uc) for uc in range(UC)]
            NS = 17
            total = OFF * (UC - 1) + NS
            for step in range(total):
                for uc in range(UC):
                    k_ = step - OFF * uc
                    if 0 <= k_ < NS:
                        all_stages[uc][k_]()


def _host_prep(inputs: np.ndarray, weights: np.ndarray):
    wbf = weights.astype(NPBF16)
    w2 = (wbf.astype(np.float32) ** 2)
    # [U,N,C,D] -> d-major [U, D, N, C] -> [UC, P, NCD]
    wT = np.ascontiguousarray(
        wbf.astype(np.float32).transpose(0, 3, 1, 2)).reshape(UC, P, NCD)
    w2T = np.ascontiguousarray(
        w2.transpose(0, 3, 1, 2)).reshape(UC, P, NCD)
    w2s = np.ascontiguousarray(w2.sum(axis=-1).reshape(UC, P, K)).astype(
        np.float32)
    a = np.ascontiguousarray(inputs.transpose(0, 2, 1))  # [B, U, C]
    abf = a.astype(NPBF16)
    a2 = abf.astype(np.float32) ** 2
    # [B, U, C] -> [UC, P, B, C] per core slice later
    return (wT.astype(NPBF16), w2T.astype(NPBF16), w2s, abf, a2)


_NC_CACHE = {}


def _get_program():
    if "p" not in _NC_CACHE:
        _NC_CACHE["p"] = build_program()
    return _NC_CACHE["p"]


def kernel(inputs: np.ndarray, weights: np.ndarray, _trace=False) -> np.ndarray:
    inputs = np.asarray(inputs, dtype=np.float32)
    weights = np.asarray(weights, dtype=np.float32)
    assert inputs.shape == (B_FULL, C, U), inputs.shape
    assert weights.shape == (U, N, C, D), weights.shape

    wT, w2T, w2s, abf, a2 = _host_prep(inputs, weights)
    nc = _get_program()
    in_maps = []
    for core in range(N_CORES):
        bs = slice(core * B_CORE, (core + 1) * B_CORE)
        # a[b,u,c] slice -> [UC, P, B_CORE*C]
        ab = abf[bs]  # [4, U, C]
        a2b = a2[bs]
        vt = np.ascontiguousarray(
            ab.reshape(B_CORE, UC, P, C).transpose(1, 2, 0, 3)).reshape(
            UC, P, B_CORE * C)
        a2t = np.ascontiguousarray(
            a2b.reshape(B_CORE, UC, P, C).transpose(1, 2, 0, 3)).reshape(
            UC, P, B_CORE * C)
        in_maps.append({
            "w": wT, "w2": w2T, "w2s": w2s,
            "vt": vt.astype(NPBF16), "a2": a2t.astype(np.float32),
        })
    res = run_bass_kernel_spmd(nc, in_maps, list(range(N_CORES)),
                               trace=_trace)
    outs = []
    for core in range(N_CORES):
        o = res.results[core]["out"]  # [UC, P, B_CORE*D*K] bf16
        o = o.reshape(UC, P, B_CORE, D, N, C).astype(np.float32)
        # -> [B_CORE, N, U, C, D]
        o = o.transpose(2, 4, 0, 1, 5, 3).reshape(B_CORE, N, U, C, D)
        outs.append(o)
    full = np.ascontiguousarray(np.concatenate(outs, axis=0))
    if _trace:
        kernel.last_exec_time_ns = res.exec_time_ns
    return full


kernel.last_exec_time_ns = None


if __name__ == "__main__":
    rng = np.random.default_rng(0)
    inputs = rng.standard_normal((B_FULL, C, U), dtype=np.float32)
    weights = rng.standard_normal((U, N, C, D), dtype=np.float32)
    out = kernel(inputs, weights)
    print("out shape", out.shape, out.dtype)
